# revision 10
# baseline (speedup 1.0000x reference)
"""Trainium2 Bass kernel for nn_ConduitHydrology: CG solve of the 5-point
Neumann Laplacian on a 2048x2048 raster, sharded by row-blocks over 8 cores.

kernel(**inputs) takes FULL inputs and returns the FULL output.

Per-call data motion over the axon tunnel is the bottleneck, so the host
uploads a single packed f16 array (b block + 2 ghost rows per core, scaled
by S to stay in f16 normal range), the jitted executable + constant
operands are cached across calls, output scratch buffers are donate-chained
from the previous call, and x comes back as f16.  gamma0 = b.b is computed
on-device and rides in slot 1 of the first iteration's AllGather.
"""
import numpy as np

import concourse.bass as bass
import concourse.bacc as bacc
import concourse.mybir as mybir
import concourse.tile as tile

F32 = mybir.dt.float32
F16 = mybir.dt.float16
I32 = mybir.dt.int32
NCORES = 8
R, C = 2048, 2048
BR = R // NCORES          # 256 rows per core
HALF = 2048
NFREE = 2 * HALF          # [128, 4096] per state tile
NITER = 100
DX = 100.0
S = 1024.0                # f16 scale for b upload
CHUNK = 512

_state = None


def _build_program(niter):
    nc = bacc.Bacc("TRN2", target_bir_lowering=False, debug=False,
                   num_devices=NCORES)

    b_in = nc.dram_tensor("bblk", [BR + 2, C], F16, kind="ExternalInput")
    t0_in = nc.dram_tensor("t0", [128, 128], F32, kind="ExternalInput")
    t1_in = nc.dram_tensor("t1", [128, 128], F32, kind="ExternalInput")
    u0_in = nc.dram_tensor("u0", [2, 128], F32, kind="ExternalInput")
    u1_in = nc.dram_tensor("u1", [2, 128], F32, kind="ExternalInput")
    gidx_in = nc.dram_tensor("gidx", [2, 1], I32, kind="ExternalInput")
    # rows 0..255: int8 quantized x; rows 256..263: the [128,32] f32
    # per-(partition,128-col-chunk) dequant scales, bitcast to int8 bytes
    x_out = nc.dram_tensor("xout", [BR + 8, C], mybir.dt.int8,
                           kind="ExternalOutput")

    ccA_in = nc.dram_tensor("ccA_in", [1, 16], F32, kind="Internal")
    ccA_out = nc.dram_tensor("ccA_out", [NCORES, 16], F32, kind="Internal",
                             addr_space="Shared")
    ccB_in = nc.dram_tensor("ccB_in", [3, HALF], F32, kind="Internal")
    ccB_out = nc.dram_tensor("ccB_out", [3 * NCORES, HALF], F32,
                             kind="Internal", addr_space="Shared")
    rg = [list(range(NCORES))]

    with tile.TileContext(nc) as tc:
        with tc.tile_pool(name="state", bufs=1) as sp, \
             tc.tile_pool(name="psumV", bufs=1, space="PSUM") as ppv, \
             tc.tile_pool(name="psumS", bufs=1, space="PSUM") as pps:
            # persistent state
            x = sp.tile([128, NFREE], F32, name="x")
            r = sp.tile([128, NFREE], F32, name="r")
            p = sp.tile([128, NFREE], F32, name="p")
            q = sp.tile([128, NFREE], F32, name="q")
            s1 = sp.tile([128, NFREE], F32, name="s1")
            bst = sp.tile([128, NFREE], F16, name="bst")
            gst = sp.tile([2, HALF], F16, name="gst")
            cm = sp.tile([128, 32], F32, name="cm")
            rc = sp.tile([128, 32], F32, name="rc")
            rscl = sp.tile([128, 32], F32, name="rscl")
            iscl = sp.tile([128, 32], F32, name="iscl")
            tq = sp.tile([128, NFREE], F32, name="tq")
            xq8 = sp.tile([128, NFREE], mybir.dt.int8, name="xq8")
            t0 = sp.tile([128, 128], F32, name="t0s")
            t1 = sp.tile([128, 128], F32, name="t1s")
            u0 = sp.tile([2, 128], F32, name="u0s")
            u1 = sp.tile([2, 128], F32, name="u1s")
            aux0 = sp.tile([2, HALF], F32, name="aux0")
            aux1 = sp.tile([2, HALF], F32, name="aux1")
            gp = sp.tile([2, HALF], F32, name="gp")
            rgp = sp.tile([2, HALF], F32, name="rgp")
            gidx = sp.tile([2, 1], I32, name="gidx")
            gam = sp.tile([1, 1], F32, name="gam")
            ones_c = sp.tile([128, 1], F32, name="ones_c")
            ones_r = sp.tile([1, 128], F32, name="ones_r")
            pq_part = sp.tile([128, 1], F32, name="pq_part")
            rr_part = sp.tile([128, 1], F32, name="rr_part")
            sqd = sp.tile([128, NFREE], F32, name="sqd")
            sqd2 = sp.tile([128, NFREE], F32, name="sqd2")
            g8 = sp.tile([1, 128], F32, name="g8")
            sc = sp.tile([1, 8], F32, name="sc")  # scalar scratch
            ab = sp.tile([128, 2], F32, name="ab")  # alpha / -alpha bcast
            bb = sp.tile([128, 1], F32, name="bb")  # beta bcast

            # ---- init ----
            nc.sync.dma_start(bst[:, 0:HALF], b_in.ap()[0:128, :])
            nc.sync.dma_start(bst[:, HALF:NFREE], b_in.ap()[128:256, :])
            nc.sync.dma_start(gst[:], b_in.ap()[256:258, :])
            nc.sync.dma_start(t0[:], t0_in.ap())
            nc.sync.dma_start(t1[:], t1_in.ap())
            nc.sync.dma_start(u0[:], u0_in.ap())
            nc.sync.dma_start(u1[:], u1_in.ap())
            nc.sync.dma_start(gidx[:], gidx_in.ap())
            nc.vector.memset(x[:], 0.0)
            nc.vector.memset(ones_c[:], 1.0)
            nc.vector.memset(ones_r[:], 1.0)
            nc.vector.memset(g8[:], 1.0)
            # f16 -> f32 casts
            nc.scalar.copy(r[:], bst[:])
            nc.vector.tensor_copy(p[:], r[:])
            nc.scalar.copy(gp[:], gst[:])
            # initial aux: ghost rows + local cross-slab rows of p(=b)
            nc.sync.dma_start(aux0[0:1, :], gp[0:1, :])
            nc.sync.dma_start(aux1[1:2, :], gp[1:2, :])
            nc.sync.dma_start(aux0[1:2, :], p[0:1, HALF:NFREE])
            nc.sync.dma_start(aux1[0:1, :], p[127:128, 0:HALF])
            # ---- gamma0 partial = b.b (rides in CC-A slot 1 of iter 1)
            nc.scalar.activation(sqd2[:], r[:],
                                 mybir.ActivationFunctionType.Square,
                                 accum_out=rr_part[:])
            red0 = pps.tile([1, 2], F32, name="red0", tag="red")
            nc.tensor.matmul(red0[:, 0:1], ones_c[:], rr_part[:],
                             start=True, stop=True)
            nc.scalar.copy(sc[:, 5:6], red0[:, 0:1])
            nc.sync.dma_start(ccA_in.ap()[0:1, 1:2], sc[:, 5:6])

            ts = [t0, t1]
            us = [u0, u1]
            auxs = [aux0, aux1]

            for it in range(niter):
                # ---- matvec q = L p ----
                for s in range(2):
                    o = s * HALF
                    ps = p[:, o:o + HALF]
                    # horizontal shifted sums with edge-column degh fix
                    nc.vector.tensor_tensor(
                        s1[:, o + 1:o + HALF - 1], ps[:, 0:HALF - 2],
                        ps[:, 2:HALF], mybir.AluOpType.add)
                    nc.vector.tensor_tensor(
                        s1[:, o:o + 1], ps[:, 0:1], ps[:, 1:2],
                        mybir.AluOpType.add)
                    nc.vector.tensor_tensor(
                        s1[:, o + HALF - 1:o + HALF], ps[:, HALF - 2:HALF - 1],
                        ps[:, HALF - 1:HALF], mybir.AluOpType.add)
                    # vertical + diagonal via PE
                    vt = ppv.tile([128, HALF], F32, name="vt", tag="vt")
                    for ch in range(0, HALF, CHUNK):
                        nc.tensor.matmul(vt[:, ch:ch + CHUNK], ts[s][:],
                                         ps[:, ch:ch + CHUNK],
                                         start=True, stop=False)
                        nc.tensor.matmul(vt[:, ch:ch + CHUNK], us[s][:],
                                         auxs[s][:, ch:ch + CHUNK],
                                         start=False, stop=True)
                    nc.vector.tensor_tensor(
                        q[:, o:o + HALF], s1[:, o:o + HALF], vt[:],
                        mybir.AluOpType.add)

                # ---- pq = p . q (accum per partition, then partition reduce)
                nc.vector.scalar_tensor_tensor(
                    sqd[:], p[:], 1.0, q[:],
                    mybir.AluOpType.mult, mybir.AluOpType.mult,
                    accum_out=pq_part[:])
                red = pps.tile([1, 2], F32, name="red", tag="red")
                nc.tensor.matmul(red[:, 0:1], ones_c[:], pq_part[:],
                                 start=True, stop=True)
                nc.scalar.copy(sc[:, 0:1], red[:, 0:1])
                nc.sync.dma_start(ccA_in.ap()[0:1, 0:1], sc[:, 0:1])
                nc.gpsimd.collective_compute(
                    "AllGather", mybir.AluOpType.bypass, replica_groups=rg,
                    ins=[ccA_in.ap()], outs=[ccA_out.ap()])
                nc.sync.dma_start(
                    g8[:], ccA_out.ap().rearrange("(o a) b -> o (a b)", o=1))
                nc.vector.tensor_reduce(
                    sc[:, 1:2],
                    g8[0:1, :].rearrange("a (c s) -> a c s", s=16)[:, :, 0:1],
                    axis=mybir.AxisListType.XY, op=mybir.AluOpType.add)
                if it == 0:
                    # gamma0 = sum of slot-1 partials
                    nc.vector.tensor_reduce(
                        gam[:],
                        g8[0:1, :].rearrange("a (c s) -> a c s",
                                             s=16)[:, :, 1:2],
                        axis=mybir.AxisListType.XY, op=mybir.AluOpType.add)
                # alpha = gam / pq ; nalpha = -alpha
                nc.vector.reciprocal(sc[:, 2:3], sc[:, 1:2])
                nc.vector.tensor_tensor(sc[:, 3:4], sc[:, 2:3], gam[:],
                                        mybir.AluOpType.mult)
                nc.vector.tensor_scalar_mul(sc[:, 4:5], sc[:, 3:4], -1.0)
                bc = pps.tile([128, 2], F32, name="bc", tag="bc")
                nc.tensor.matmul(bc[:], ones_r[:], sc[0:1, 3:5],
                                 start=True, stop=True)
                nc.scalar.copy(ab[:], bc[:])

                # ---- r -= alpha q ----
                nc.vector.scalar_tensor_tensor(
                    r[:], q[:], ab[:, 1:2], r[:],
                    mybir.AluOpType.mult, mybir.AluOpType.add)

                # ---- rr partial on ACT; stage CC-B (rr + r boundary rows)
                nc.scalar.activation(sqd2[:], r[:],
                                     mybir.ActivationFunctionType.Square,
                                     accum_out=rr_part[:])
                red2 = pps.tile([1, 2], F32, name="red2", tag="red")
                nc.tensor.matmul(red2[:, 0:1], ones_c[:], rr_part[:],
                                 start=True, stop=True)
                nc.scalar.copy(sc[:, 5:6], red2[:, 0:1])
                nc.sync.dma_start(ccB_in.ap()[0:1, 0:1], sc[:, 5:6])
                nc.sync.dma_start(ccB_in.ap()[1:2, :], r[0:1, 0:HALF])
                nc.sync.dma_start(ccB_in.ap()[2:3, :], r[127:128, HALF:NFREE])
                nc.gpsimd.collective_compute(
                    "AllGather", mybir.AluOpType.bypass, replica_groups=rg,
                    ins=[ccB_in.ap()], outs=[ccB_out.ap()])

                # ---- x += alpha p (overlaps CC-B) ----
                nc.vector.scalar_tensor_tensor(
                    x[:], p[:], ab[:, 0:1], x[:],
                    mybir.AluOpType.mult, mybir.AluOpType.add)

                # ---- gamma_new, beta ----
                nc.sync.dma_start(
                    g8[:, 0:24],
                    ccB_out.ap()[:, 0:1].rearrange("(o a) b -> o (a b)", o=1))
                nc.vector.tensor_reduce(
                    sc[:, 6:7],
                    g8[0:1, 0:24].rearrange("a (c s) -> a c s", s=3)[:, :, 0:1],
                    axis=mybir.AxisListType.XY, op=mybir.AluOpType.add)
                nc.vector.reciprocal(sc[:, 7:8], gam[:])
                nc.vector.tensor_tensor(sc[:, 0:1], sc[:, 6:7], sc[:, 7:8],
                                        mybir.AluOpType.mult)
                nc.vector.tensor_copy(gam[:], sc[:, 6:7])
                bc2 = pps.tile([128, 1], F32, name="bc2", tag="bc")
                nc.tensor.matmul(bc2[:], ones_r[:], sc[0:1, 0:1],
                                 start=True, stop=True)
                nc.scalar.copy(bb[:], bc2[:])

                # ---- p = r + beta p ----
                nc.vector.scalar_tensor_tensor(
                    p[:], p[:], bb[:], r[:],
                    mybir.AluOpType.mult, mybir.AluOpType.add)

                if it < niter - 1:
                    # ---- ghost p rows: gp = beta*gp + r_ghost ----
                    nc.gpsimd.indirect_dma_start(
                        out=rgp[:], out_offset=None, in_=ccB_out.ap(),
                        in_offset=bass.IndirectOffsetOnAxis(ap=gidx[:, :1],
                                                            axis=0))
                    nc.vector.scalar_tensor_tensor(
                        gp[:], gp[:], bb[0:2, :], rgp[:],
                        mybir.AluOpType.mult, mybir.AluOpType.add)
                    # refresh aux tiles
                    nc.sync.dma_start(aux0[0:1, :], gp[0:1, :])
                    nc.sync.dma_start(aux1[1:2, :], gp[1:2, :])
                    nc.sync.dma_start(aux0[1:2, :], p[0:1, HALF:NFREE])
                    nc.sync.dma_start(aux1[0:1, :], p[127:128, 0:HALF])

            # ---- int8 per-chunk quantization of x ----
            # round-to-nearest via the f32 magic-number trick so the final
            # int8 cast sees an exactly-integral f32 (cast mode irrelevant)
            MAGIC = 12582912.0  # 1.5 * 2**23
            nc.vector.tensor_reduce(
                cm[:], x[:].rearrange("a (c w) -> a c w", w=128),
                axis=mybir.AxisListType.X, op=mybir.AluOpType.max,
                apply_absolute_value=True)
            nc.vector.tensor_scalar_add(cm[:], cm[:], 1e-20)
            nc.vector.tensor_scalar_mul(iscl[:], cm[:], 1.0 / 127.0)
            nc.vector.reciprocal(rc[:], cm[:])
            nc.vector.tensor_scalar_mul(rscl[:], rc[:], 127.0)
            for c in range(32):
                nc.scalar.activation(
                    tq[:, c * 128:(c + 1) * 128], x[:, c * 128:(c + 1) * 128],
                    mybir.ActivationFunctionType.Copy,
                    bias=MAGIC, scale=rscl[:, c:c + 1])
            nc.scalar.activation(xq8[:], tq[:],
                                 mybir.ActivationFunctionType.Copy,
                                 bias=-MAGIC, scale=1.0)
            nc.sync.dma_start(x_out.ap()[0:128, :], xq8[:, 0:HALF])
            nc.sync.dma_start(x_out.ap()[128:256, :], xq8[:, HALF:NFREE])
            nc.sync.dma_start(
                x_out.ap()[256:264, :].rearrange("a (b c) -> (a b) c", b=16),
                iscl[:].bitcast(mybir.dt.int8))

    nc.compile()
    return nc


def _host_prep_b16(conduit_size, discharge, geometric_gradient):
    """Full-grid RHS b (scaled by S), quantized to f16, packed per-core as
    [8*(BR+2), C]: 256 block rows then 2 ghost rows (north, south).

    Interior uses the centered-difference identity
    b = 0.5*(gn[c+1]-gn[c-1]) + 0.5*(gn[r+1]-gn[r-1]); the status-masked
    boundary links only affect a 2-wide frame, patched exactly below.
    """
    cs = np.asarray(conduit_size, dtype=np.float32).reshape(R, C)
    dc = np.asarray(discharge, dtype=np.float32).reshape(R, C)
    gg = np.asarray(geometric_gradient, dtype=np.float32).reshape(R, C)

    HS = np.float32(0.5 * S)
    # gn = (dc * 0.0405 * cs**1.25)**2 * HS, in-place (4 passes)
    gn = cs ** np.float32(1.25)
    gn *= dc
    np.square(gn, out=gn)
    gn *= np.float32(0.0405 * 0.0405 * 0.5 * S)

    b = np.empty((R, C), dtype=np.float32)
    np.subtract(gn[2:-2, 3:-1], gn[2:-2, 1:-3], out=b[2:-2, 2:-2])
    b[2:-2, 2:-2] += gn[3:-1, 2:-2]
    b[2:-2, 2:-2] -= gn[1:-3, 2:-2]

    # --- top band (rows 0,1) ---
    gh0 = HS * (gg[0, :-1] + gg[0, 1:])
    gh1 = gn[1, :-1] + gn[1, 1:]
    gh1[0] = HS * (gg[1, 0] + gg[1, 1])
    gh1[-1] = HS * (gg[1, -2] + gg[1, -1])
    gv0 = HS * (gg[0, :] + gg[1, :])
    gv1 = gn[1, :] + gn[2, :]
    gv1[0] = HS * (gg[1, 0] + gg[2, 0])
    gv1[-1] = HS * (gg[1, -1] + gg[2, -1])
    b[0, :] = gv0
    b[0, :-1] += gh0
    b[0, 1:] -= gh0
    b[1, :] = gv1 - gv0
    b[1, :-1] += gh1
    b[1, 1:] -= gh1
    # --- bottom band (rows R-2, R-1) ---
    ghm = gn[-2, :-1] + gn[-2, 1:]
    ghm[0] = HS * (gg[-2, 0] + gg[-2, 1])
    ghm[-1] = HS * (gg[-2, -2] + gg[-2, -1])
    ghl = HS * (gg[-1, :-1] + gg[-1, 1:])
    gvp = gn[-3, :] + gn[-2, :]
    gvp[0] = HS * (gg[-3, 0] + gg[-2, 0])
    gvp[-1] = HS * (gg[-3, -1] + gg[-2, -1])
    gvl = HS * (gg[-2, :] + gg[-1, :])
    b[-2, :] = gvl - gvp
    b[-2, :-1] += ghm
    b[-2, 1:] -= ghm
    b[-1, :] = -gvl
    b[-1, :-1] += ghl
    b[-1, 1:] -= ghl
    # --- left/right bands (cols 0,1 and C-2,C-1; rows 2..R-3) ---
    rs = slice(2, R - 2)
    ghc0 = HS * (gg[rs, 0] + gg[rs, 1])
    b[rs, 0] = ghc0 + HS * (gg[3:R - 1, 0] - gg[1:R - 3, 0])
    b[rs, 1] = (gn[rs, 1] + gn[rs, 2]) - ghc0 \
        + (gn[3:R - 1, 1] - gn[1:R - 3, 1])
    ghc1 = HS * (gg[rs, -2] + gg[rs, -1])
    b[rs, -2] = ghc1 - (gn[rs, -3] + gn[rs, -2]) \
        + (gn[3:R - 1, -2] - gn[1:R - 3, -2])
    b[rs, -1] = -ghc1 + HS * (gg[3:R - 1, -1] - gg[1:R - 3, -1])

    pad = np.zeros((NCORES, BR + 2, C), dtype=np.float16)
    pad[:, 0:BR, :] = b.reshape(NCORES, BR, C)
    pad[1:, 256, :] = pad[:-1, 255, :]             # north ghost = row i*BR-1
    pad[:-1, 257, :] = pad[1:, 0, :]               # south ghost = row (i+1)*BR
    return pad.reshape(NCORES * (BR + 2), C), gg


def _make_consts():
    t0g = np.zeros((NCORES, 128, 128), dtype=np.float32)
    t1g = np.zeros((NCORES, 128, 128), dtype=np.float32)
    u0g = np.zeros((NCORES, 2, 128), dtype=np.float32)
    u1g = np.zeros((NCORES, 2, 128), dtype=np.float32)
    gig = np.zeros((NCORES, 2, 1), dtype=np.int32)
    for i in range(NCORES):
        for t, base in ((t0g[i], i * BR), (t1g[i], i * BR + 128)):
            for j in range(128):
                grow = base + j
                degv = 2 - (1 if grow == 0 else 0) - (1 if grow == R - 1 else 0)
                t[j, j] = -(degv + 2)
                if j > 0:
                    t[j, j - 1] = 1.0
                if j < 127:
                    t[j, j + 1] = 1.0
        u0g[i, 0, 0] = 0.0 if i == 0 else 1.0
        u0g[i, 1, 127] = 1.0
        u1g[i, 0, 0] = 1.0
        u1g[i, 1, 127] = 0.0 if i == NCORES - 1 else 1.0
        gig[i, 0, 0] = 3 * (i - 1) + 2 if i > 0 else 1
        gig[i, 1, 0] = 3 * (i + 1) + 1 if i < NCORES - 1 else 1
    return {
        "t0": t0g.reshape(NCORES * 128, 128),
        "t1": t1g.reshape(NCORES * 128, 128),
        "u0": u0g.reshape(NCORES * 2, 128),
        "u1": u1g.reshape(NCORES * 2, 128),
        "gidx": gig.reshape(NCORES * 2, 1),
    }


def _setup():
    """Compile the program, build the cached jitted runner + device-resident
    constant operands.  One-time cost, excluded from warm-call timing."""
    import jax
    from jax.sharding import Mesh, PartitionSpec as P, NamedSharding
    from jax.experimental.shard_map import shard_map
    from concourse.bass2jax import (_bass_exec_p, install_neuronx_cc_hook,
                                    partition_id_tensor)

    nc = _build_program(NITER)
    install_neuronx_cc_hook()

    devs = jax.devices()[:NCORES]
    mesh = Mesh(np.asarray(devs), ("core",))
    sh = NamedSharding(mesh, P("core"))

    partition_name = (nc.partition_id_tensor.name
                      if nc.partition_id_tensor else None)
    in_names, out_names, out_avals = [], [], []
    for alloc in nc.m.functions[0].allocations:
        if not isinstance(alloc, mybir.MemoryLocationSet):
            continue
        name = alloc.memorylocations[0].name
        if alloc.kind == "ExternalInput":
            if name != partition_name:
                in_names.append(name)
        elif alloc.kind == "ExternalOutput":
            out_names.append(name)
            shape = tuple(alloc.tensor_shape)
            dtype = mybir.dt.np(alloc.dtype)
            out_avals.append(jax.core.ShapedArray(shape, dtype))
    n_params = len(in_names)
    n_outs = len(out_avals)
    in_names_full = in_names + out_names + (
        [partition_name] if partition_name else [])
    donate = tuple(range(n_params, n_params + n_outs))

    def _body(*args):
        operands = list(args)
        if partition_name is not None:
            operands.append(partition_id_tensor())
        outs = _bass_exec_p.bind(
            *operands, out_avals=tuple(out_avals),
            in_names=tuple(in_names_full), out_names=tuple(out_names),
            lowering_input_output_aliases=(), sim_require_finite=True,
            sim_require_nnan=True, nc=nc)
        return tuple(outs)

    sharded = jax.jit(
        shard_map(_body, mesh=mesh, in_specs=(P("core"),) * (n_params + n_outs),
                  out_specs=(P("core"),) * n_outs, check_rep=False),
        donate_argnums=donate, keep_unused=True)

    consts_np = _make_consts()
    consts_dev = {k: jax.device_put(v, sh) for k, v in consts_np.items()}
    for v in consts_dev.values():
        v.block_until_ready()

    zshapes = [(NCORES * a.shape[0], *a.shape[1:]) for a in out_avals]
    zdtypes = [a.dtype for a in out_avals]
    zeros_fn = jax.jit(
        lambda: tuple(jax.numpy.zeros(s, d) for s, d in zip(zshapes, zdtypes)),
        out_shardings=tuple(sh for _ in zshapes))

    return {
        "sharded": sharded,
        "in_names": in_names,
        "consts_dev": consts_dev,
        "zeros_fn": zeros_fn,
        "prev_out": None,
    }


def kernel(conduit_size, discharge, geometric_gradient, nrows, ncols):
    global _state
    b16pad, gg = _host_prep_b16(conduit_size, discharge, geometric_gradient)

    if _state is None:
        _state = _setup()
    st = _state

    arg_by_name = dict(st["consts_dev"])
    arg_by_name["bblk"] = b16pad
    args = [arg_by_name[nm] for nm in st["in_names"]]

    if st["prev_out"] is None:
        scratch = st["zeros_fn"]()
    else:
        scratch = st["prev_out"]
    outs = st["sharded"](*args, *scratch)
    raw = np.asarray(outs[0])                       # [8*264, 2048] int8
    st["prev_out"] = outs

    raw = raw.reshape(NCORES, BR + 8, C)
    iscl = np.ascontiguousarray(raw[:, BR:BR + 8, :]).view(np.float32)
    iscl = iscl.reshape(NCORES, 128, 2, 16).transpose(0, 2, 1, 3)
    iscl = iscl * np.float32(DX / S)                # [core, slab, j, chunk]
    xf = raw[:, 0:BR, :].astype(np.float32)
    xf = xf.reshape(NCORES, 2, 128, 16, 128)
    xf *= iscl[..., None]
    xfr = xf.reshape(R, C)
    np.subtract(gg, xfr, out=xfr)
    return xfr.reshape(-1)
